# revision 1
# baseline (speedup 1.0000x reference)
"""DenoiseGAT Trainium2 kernel: 8-core data-parallel over polygons (cycle graphs).

Per core: 256 polygons x 64 nodes = 16384 nodes. Activations as h^T
(features x nodes, bf16), 256-row tensors stored as [128, 2, n] tiles
(half index on the free dim). Attention: scores via block-diag a-matmul;
softmax in poly-partition block layout; alpha replicated to feature rows
via DRAM-staged broadcast DMA; neighbor combine via +-1 shifted tensor
ops (shifts stay inside 64-node polygons).
"""

import numpy as np
import ml_dtypes
from contextlib import ExitStack

import concourse.bass as bass
import concourse.tile as tile
import concourse.tile_utils as tile_utils
from concourse import bacc, mybir
from concourse.bass_utils import run_bass_kernel_spmd

tile_utils.max_sbuf_usage = 208 * 1024

F32 = mybir.dt.float32
BF16 = mybir.dt.bfloat16
ALU = mybir.AluOpType
ACTF = mybir.ActivationFunctionType

NCORES = 8
B, V = 2048, 64
HID, TDIM = 256, 128
BC = B // NCORES            # 256 polygons / core
N = BC * V                  # 16384 nodes / core
NT = 512                    # matmul node tile
SCH = 8192                  # softmax chunk = 128 polys
CCH = 1024                  # combine subchunk = 16 polys


def _ablk(asrc, atgt):
    NH, FO = asrc.shape
    out = np.zeros((NH * FO, 2 * NH), np.float32)
    for h in range(NH):
        out[h * FO:(h + 1) * FO, h] = asrc[h]
        out[h * FO:(h + 1) * FO, NH + h] = atgt[h]
    return out


def _bf(a):
    return np.ascontiguousarray(np.asarray(a, np.float32).astype(ml_dtypes.bfloat16))


def _f32(a):
    return np.ascontiguousarray(np.asarray(a, np.float32))


def _poly(ap, v=V):
    return ap.rearrange("p (g v) -> p g v", v=v)


def build(weights):
    nc = bacc.Bacc("TRN2", target_bir_lowering=False, debug=False,
                   enable_asserts=False, num_devices=NCORES)
    w = weights

    def inl(name, arr):
        return nc.inline_tensor(np.ascontiguousarray(arr), name=name).ap()

    half = TDIM // 2
    freqs = np.exp(-np.log(10000.0) * np.arange(half, dtype=np.float32) / (half - 1))
    fr2 = np.stack([np.concatenate([freqs, freqs]),
                    np.concatenate([np.zeros(half, np.float32),
                                    np.full(half, np.pi / 2, np.float32)])])
    ph = np.arange(V, dtype=np.float32) * (2 * np.pi / V)
    posT = np.tile(np.stack([np.sin(ph), np.cos(ph), np.sin(2 * ph), np.cos(2 * ph)]), (1, BC))

    def half3(a):
        """(256, X) host -> (128, 2, X) so tile[:, j, :] == rows 128j:128j+128."""
        a = np.asarray(a)
        return np.ascontiguousarray(a.reshape(2, 128, a.shape[1]).transpose(1, 0, 2))

    W0 = _f32(w["W0"]); sk0 = _f32(w["skip0"]); ab0 = _ablk(_f32(w["asrc0"]), _f32(w["atgt0"]))
    c_fr2 = inl("fr2", fr2.astype(np.float32))
    c_tW = inl("tW", _f32(w["tW"]))
    c_tb = inl("tb", _f32(w["tb"]).reshape(-1, 1))
    c_posT = inl("posT", _bf(posT))
    c_Wsum0t = inl("Wsum0t", W0[6:] + sk0[6:])          # (128, 256)
    c_W0ab = inl("W0ab", W0[6:] @ ab0)                  # (128, 8)
    c_W0f = inl("W0f", _bf(np.concatenate([W0[:6], sk0[:6]], 1)))   # (6, 512)
    c_ab0 = inl("ab0", half3(_bf(ab0)))                 # (128, 2, 8)
    c_b0 = inl("b0c", half3(_f32(w["b0"]).reshape(-1, 1)))
    c_eye8 = inl("eye8", np.eye(8, dtype=np.float32))
    LW, LAB, LB = {}, {}, {}
    for i in (1, 2):
        LW[i] = inl(f"W{i}f", half3(_bf(np.concatenate([_f32(w[f"W{i}"]), _f32(w[f"skip{i}"])], 1))))
        LAB[i] = inl(f"ab{i}f", half3(_bf(_ablk(_f32(w[f"asrc{i}"]), _f32(w[f"atgt{i}"])))))
        LB[i] = inl(f"b{i}c", half3(_f32(w[f"b{i}"]).reshape(-1, 1)))
    c_W3 = inl("W3f", half3(_bf(_f32(w["W3"]))))
    c_ab3 = inl("ab3f", half3(_bf(_ablk(_f32(w["asrc3"]), _f32(w["atgt3"])))))
    c_b3 = inl("b3c", half3(_f32(w["b3"]).reshape(-1, 1)))
    c_h1W = inl("h1Wf", half3(_bf(_f32(w["h1W"]))))
    c_h1b = inl("h1bc", half3(_f32(w["h1b"]).reshape(-1, 1)))
    c_h2W = inl("h2Wf", half3(_bf(_f32(w["h2W"]))))
    c_h2b = inl("h2bc", _f32(w["h2b"]).reshape(-1, 1))

    xT = nc.dram_tensor("xT", [2, N], BF16, kind="ExternalInput").ap()
    tp = nc.dram_tensor("tp", [2, BC], F32, kind="ExternalInput").ap()
    yT = nc.dram_tensor("yT", [2, N], F32, kind="ExternalOutput").ap()

    with tile.TileContext(nc) as tc, ExitStack() as ctx:
        P = ctx.enter_context(tc.tile_pool(name="pers", bufs=1))
        WP = ctx.enter_context(tc.tile_pool(name="wts", bufs=1))
        DR = ctx.enter_context(tc.tile_pool(name="dram", bufs=1, space="DRAM"))
        PS = ctx.enter_context(tc.tile_pool(name="ps", bufs=5, space="PSUM"))
        PSC = ctx.enter_context(tc.tile_pool(name="pssc", bufs=3, space="PSUM"))
        SM = ctx.enter_context(tc.tile_pool(name="sm", bufs=1))
        CB = ctx.enter_context(tc.tile_pool(name="cb", bufs=1))
        SK = ctx.enter_context(tc.tile_pool(name="sk", bufs=2))
        PJ = ctx.enter_context(tc.tile_pool(name="pj", bufs=1))

        h = P.tile([128, 2, N], BF16, tag="h")
        tembT = P.tile([TDIM, BC], F32, tag="tembT")
        G0T = P.tile([128, 2, BC], F32, tag="G0T")
        s0gT2 = P.tile([128, 2, 8], F32, tag="s0gT2")

        def load(c_ap, tag):
            t = WP.tile(list(c_ap.shape), c_ap.dtype, tag=tag)
            nc.sync.dma_start(t[:], c_ap)
            return t

        t_eye8 = load(c_eye8, "eye8")
        t_fr2 = load(c_fr2, "fr2")
        t_tp = load(tp, "tp")
        ps_te = PSC.tile([TDIM, BC], F32, tag="psA")
        nc.tensor.matmul(ps_te[:], t_fr2[:], t_tp[:], start=True, stop=True)
        te_m = SM.tile([TDIM, BC], F32, tag="Sblk")
        te_q = SM.tile([TDIM, BC], mybir.dt.int32, tag="den")
        nc.vector.tensor_scalar(te_q[:], ps_te[:], float(1.0 / (2 * np.pi)), None, op0=ALU.mult)
        te_qf = SM.tile([TDIM, BC], F32, tag="rd")
        nc.vector.tensor_copy(te_qf[:], te_q[:])
        nc.vector.scalar_tensor_tensor(te_m[:], te_qf[:], float(-2 * np.pi), ps_te[:],
                                       op0=ALU.mult, op1=ALU.add)
        te_s = SM.tile([TDIM, BC], F32, tag="E")
        nc.scalar.activation(te_s[:], te_m[:], ACTF.Sin)
        t_tW = load(c_tW, "tW")
        t_tb = load(c_tb, "tb")
        ps_tm = PSC.tile([TDIM, BC], F32, tag="psA")
        nc.tensor.matmul(ps_tm[:], t_tW[:], te_s[:], start=True, stop=True)
        nc.scalar.activation(tembT[:], ps_tm[:], ACTF.Silu, bias=t_tb[:])

        t_Ws0 = load(c_Wsum0t, "Ws0")
        for m in range(2):
            ps_g = PSC.tile([128, BC], F32, tag="psA")
            nc.tensor.matmul(ps_g[:], t_Ws0[:, m * 128:(m + 1) * 128], tembT[:],
                             start=True, stop=True)
            nc.vector.tensor_copy(G0T[:, m, :], ps_g[:])
        t_W0ab = load(c_W0ab, "W0ab")
        ps_sg = PSC.tile([8, BC], F32, tag="psA")
        nc.tensor.matmul(ps_sg[:], t_W0ab[:], tembT[:], start=True, stop=True)
        s0g = SM.tile([8, BC], F32, tag="EX")
        nc.vector.tensor_copy(s0g[:], ps_sg[:])
        for m in range(2):
            ps_t = PSC.tile([128, 8], F32, tag="psA")
            nc.tensor.matmul(ps_t[:], s0g[:, m * 128:(m + 1) * 128], t_eye8[:],
                             is_transpose=True, start=True, stop=True)
            nc.vector.tensor_copy(s0gT2[:, m, :], ps_t[:])

        h0loc = PJ.tile([6, SCH], BF16, tag="h0loc")
        TT = nc.vector.tensor_tensor
        GT = nc.gpsimd.tensor_tensor
        STT = nc.vector.scalar_tensor_tensor

        def layer(li, FIN, R, FO, c_w, c_ab, c_bias, first_layer):
            FOW = R * FO                 # 256
            nmt = (2 * FOW if li < 3 else FOW) // 128   # output 128-blocks
            kt = (FIN + 127) // 128
            t_w = load(c_w, f"w{li}")    # (6,512) L0 else (128, 2, 512|256)
            t_ab = load(c_ab, f"ab{li}")  # (128, 2, 2R)
            t_b = load(c_bias, f"b{li}")  # (128, 2, 1)
            a_dram = DR.tile([3 * R, N], BF16, tag="a_dram")
            sc_dram = DR.tile([2 * R, N], BF16, tag="sc_dram")

            def lhs_w(k, m):
                kk = min(128, FIN - k * 128)
                if first_layer:
                    return t_w[0:kk, m * 128:(m + 1) * 128]
                return t_w[0:kk, k, m * 128:(m + 1) * 128]

            for ch in range(N // SCH):
                u0 = ch * SCH
                if first_layer:
                    nc.sync.dma_start(h0loc[0:2, :], xT[:, u0:u0 + SCH])
                    nc.sync.dma_start(h0loc[2:6, :], c_posT[:, u0:u0 + SCH])
                projc = PJ.tile([128, 2, SCH], BF16, tag="projc")
                skc = None
                if li < 3:
                    skc = PJ.tile([128, 2, SCH], BF16, tag="skc")
                scT = SM.tile([128, SCH // 4], BF16, tag="scT")

                for it in range(SCH // NT):
                    u = it * NT
                    pst = [PS.tile([128, NT], F32, tag="mm", name=f"mm{_m}") for _m in range(nmt)]
                    for m in range(nmt):
                        for k in range(kt):
                            kk = min(128, FIN - k * 128)
                            rhs = (h0loc[0:kk, u:u + NT] if first_layer
                                   else h[0:kk, k, u0 + u:u0 + u + NT])
                            nc.tensor.matmul(pst[m][:], lhs_w(k, m), rhs,
                                             start=(k == 0), stop=(k == kt - 1))
                    for m in range(nmt):
                        if m < FOW // 128:
                            nc.scalar.activation(projc[:, m, u:u + NT], pst[m][:], ACTF.Copy)
                        else:
                            nc.scalar.activation(skc[:, m - 2, u:u + NT], pst[m][:],
                                                 ACTF.Identity, bias=t_b[:, m - 2, :])
                    s = it % 4
                    if s == 0:
                        scp = PSC.tile([128, NT], F32, tag="psA")
                    for k in range(2):
                        nc.tensor.matmul(scp[32 * s:32 * s + 2 * R, :], t_ab[:, k, :],
                                         projc[:, k, u:u + NT], start=(k == 0), stop=(k == 1),
                                         tile_position=(0, 32 * s))
                    if s == 3:
                        g = it // 4
                        nc.scalar.activation(scT[:, g * NT:(g + 1) * NT], scp[:], ACTF.Copy)

                scd = sc_dram[:, u0:u0 + SCH].rearrange("r (cb s w) -> r cb s w", s=4, w=NT)
                for s in range(4):
                    src = scT[32 * s:32 * s + 2 * R, :].rearrange("p (cb w) -> p cb w", w=NT)
                    nc.sync.dma_start(scd[:, :, s, :], src)
                S = SM.tile([128, 2 * R * V], BF16, tag="Sblk")
                src = sc_dram[:, u0:u0 + SCH].rearrange("r (p v) -> p r v", v=V)
                nc.sync.dma_start(S[:].rearrange("p (r v) -> p r v", v=V), src)

                if first_layer:
                    gb = s0gT2[:, ch, :].unsqueeze(2).to_broadcast((128, 2 * R, V))
                    Sv = S[:].rearrange("p (r v) -> p r v", v=V)
                    TT(Sv, Sv, gb, op=ALU.add)

                E = SM.tile([128, 3 * R * V], BF16, tag="E")
                Sv = S[:].rearrange("p (r v) -> p r v", v=V)
                Ssrc, Stgt = Sv[:, 0:R, :], Sv[:, R:2 * R, :]
                Ev = E[:].rearrange("p (k r v) -> p k r v", k=3, v=V)
                TT(Ev[:, 0, :, 1:], Ssrc[:, :, :V - 1], Stgt[:, :, 1:], op=ALU.add)
                TT(Ev[:, 0, :, 0:1], Ssrc[:, :, V - 1:], Stgt[:, :, 0:1], op=ALU.add)
                TT(Ev[:, 1, :, :], Ssrc, Stgt, op=ALU.add)
                TT(Ev[:, 2, :, :V - 1], Ssrc[:, :, 1:], Stgt[:, :, :V - 1], op=ALU.add)
                TT(Ev[:, 2, :, V - 1:], Ssrc[:, :, 0:1], Stgt[:, :, V - 1:], op=ALU.add)
                STT(E[:], E[:], 0.2, E[:], op0=ALU.mult, op1=ALU.max)
                EX = SM.tile([128, 3 * R * V], BF16, tag="EX")
                nc.scalar.activation(EX[:], E[:], ACTF.Exp)
                den = SM.tile([128, R * V], F32, tag="den")
                TT(den[:], EX[:, 0:R * V], EX[:, R * V:2 * R * V], op=ALU.add)
                TT(den[:], den[:], EX[:, 2 * R * V:], op=ALU.add)
                rd = SM.tile([128, R * V], F32, tag="rd")
                nc.vector.reciprocal(rd[:], den[:])
                ab_blk = SM.tile([128, 3 * R * V], BF16, tag="ab_blk")
                for k in range(3):
                    TT(ab_blk[:, k * R * V:(k + 1) * R * V],
                       EX[:, k * R * V:(k + 1) * R * V], rd[:], op=ALU.mult)
                nc.sync.dma_start(
                    a_dram[:, u0:u0 + SCH].rearrange("j (p v) -> p j v", v=V),
                    ab_blk[:].rearrange("p (j v) -> p j v", v=V))

                blk = min(FO, 128)
                for sc in range(SCH // CCH):
                    v0 = sc * CCH
                    span = slice(u0 + v0, u0 + v0 + CCH)
                    af = [CB.tile([128, 2, CCH], BF16, tag=f"af{k}", name=f"af{k}") for k in range(3)]
                    for k in range(3):
                        for b0 in range(0, FOW, blk):
                            hh = b0 // FO
                            src = a_dram[k * R + hh:k * R + hh + 1, span]
                            nc.sync.dma_start(
                                af[k][b0 % 128:b0 % 128 + blk, b0 // 128, :],
                                src.to_broadcast((blk, CCH)))
                    C1 = CB.tile([128, 2, CCH], BF16, tag="C1")
                    C2 = CB.tile([128, 2, CCH], BF16, tag="C2")
                    C4 = CB.tile([128, 2, CCH], BF16, tag="C4")
                    for ht in range(2):
                        pjv = _poly(projc[:, ht, v0:v0 + CCH])
                        a0 = _poly(af[1][:, ht, :]); ap1 = _poly(af[2][:, ht, :])
                        am1 = _poly(af[0][:, ht, :])
                        c1 = _poly(C1[:, ht, :]); c2 = _poly(C2[:, ht, :]); c4 = _poly(C4[:, ht, :])
                        TT(c1, a0, pjv, op=ALU.mult)
                        GT(c2[:, :, :V - 1], ap1[:, :, :V - 1], pjv[:, :, 1:], op=ALU.mult)
                        GT(c2[:, :, V - 1:], ap1[:, :, V - 1:], pjv[:, :, 0:1], op=ALU.mult)
                        TT(c4[:, :, 1:], am1[:, :, 1:], pjv[:, :, :V - 1], op=ALU.mult)
                        TT(c4[:, :, 0:1], am1[:, :, 0:1], pjv[:, :, V - 1:], op=ALU.mult)
                    C3 = CB.tile([128, 2, CCH], BF16, tag="C3")
                    TT(C3[:], C1[:], C4[:], op=ALU.add)
                    pre = CB.tile([128, 2, CCH], BF16, tag="pre")
                    GT(pre[:], C3[:], C2[:], op=ALU.add)
                    if li < 3:
                        GT(pre[:], pre[:], skc[:, :, v0:v0 + CCH], op=ALU.add)
                        if first_layer:
                            g0 = (u0 + v0) // V
                            for ht in range(2):
                                gbh = G0T[:, ht, g0:g0 + CCH // V].unsqueeze(2).to_broadcast(
                                    (128, CCH // V, V))
                                pvh = _poly(pre[:, ht, :])
                                TT(pvh, pvh, gbh, op=ALU.add)
                        mn = CB.tile([128, 2, CCH], BF16, tag="C1")
                        nc.vector.tensor_scalar(mn[:], pre[:], 0.0, None, op0=ALU.min)
                        ex = CB.tile([128, 2, CCH], BF16, tag="C2")
                        nc.scalar.activation(ex[:], mn[:], ACTF.Exp)
                        rl = CB.tile([128, 2, CCH], BF16, tag="C4")
                        nc.vector.tensor_scalar(rl[:], pre[:], 0.0, None, op0=ALU.max)
                        STT(h[:, :, span], ex[:], -1.0, rl[:], op0=ALU.add, op1=ALU.add)
                    else:
                        out3 = CB.tile([128, 2, CCH], BF16, tag="C1")
                        for ht in range(2):
                            STT(out3[:, ht, :], pre[:, ht, :], t_b[:, ht, :],
                                h[:, ht, span], op0=ALU.add, op1=ALU.add)
                        nc.vector.tensor_copy(h[:, :, span], out3[:])

        layer(0, 6, 4, 64, c_W0f, c_ab0, c_b0, True)
        layer(1, 256, 4, 64, LW[1], LAB[1], LB[1], False)
        layer(2, 256, 4, 64, LW[2], LAB[2], LB[2], False)
        layer(3, 256, 1, 256, c_W3, c_ab3, c_b3, False)

        t_h1W = load(c_h1W, "h1W")
        t_h1b = load(c_h1b, "h1b")
        t_h2W = load(c_h2W, "h2W")
        t_h2b = load(c_h2b, "h2b")
        for it in range(N // NT):
            u = it * NT
            pst = [PS.tile([128, NT], F32, tag="mm", name=f"mmh{_m}") for _m in range(2)]
            for m in range(2):
                for k in range(2):
                    nc.tensor.matmul(pst[m][:], t_h1W[:, k, m * 128:(m + 1) * 128],
                                     h[:, k, u:u + NT], start=(k == 0), stop=(k == 1))
            h5 = CB.tile([128, 2, NT], BF16, tag="h5")
            for m in range(2):
                nc.scalar.activation(h5[:, m, :], pst[m][:], ACTF.Silu, bias=t_h1b[:, m, :])
            ps2 = PSC.tile([2, NT], F32, tag="psA")
            for k in range(2):
                nc.tensor.matmul(ps2[:], t_h2W[:, k, :], h5[:, k, :],
                                 start=(k == 0), stop=(k == 1))
            yst = SK.tile([2, NT], F32, tag="yst")
            nc.vector.tensor_scalar(yst[:], ps2[:], t_h2b[:], None, op0=ALU.add)
            nc.sync.dma_start(yT[:, u:u + NT], yst[:])

    nc.compile()
    return nc


def kernel(**inputs):
    x = np.asarray(inputs["x"], np.float32)
    t = np.asarray(inputs["t"])
    nc = build(inputs)
    in_maps = []
    for c in range(NCORES):
        xs = x[c * BC:(c + 1) * BC]
        xTs = np.ascontiguousarray(xs.reshape(N, 2).T).astype(ml_dtypes.bfloat16)
        ts = t[c * BC:(c + 1) * BC].astype(np.float32)
        tps = np.ascontiguousarray(np.stack([ts, np.ones_like(ts)]))
        in_maps.append({"xT": xTs, "tp": tps})
    res = run_bass_kernel_spmd(nc, in_maps, core_ids=list(range(NCORES)))
    outs = []
    for c in range(NCORES):
        yTs = res.results[c]["yT"]
        outs.append(yTs.T.reshape(BC, 2 * V).astype(np.float32))
    return np.concatenate(outs, 0)



# revision 14
# speedup vs baseline: 1.6985x; 1.6985x over previous
"""DenoiseGAT Trainium2 kernel: 8-core data-parallel over polygons (cycle graphs).

Per core: 256 polygons x 64 nodes = 16384 nodes. Activations h^T
(features x nodes, bf16) as [128, 2, n] tiles. Per layer, two passes per
8192-node chunk: (A) attention scores straight from h via folded
W@a_block weights, softmax in poly-partition layout; (B) proj matmuls
into a per-poly padded layout, alpha replicated to feature rows via
broadcast DMA, combine as packed-bf16 TT ops split across DVE/Pool.
h is stored as ELU+1; the -1 is folded into the next layer's weights
via column sums (applied as ACT copy-out biases). L0's time-embedding
term enters through a K=8 indicator matmul into PSUM.
"""

import numpy as np
import ml_dtypes
from contextlib import ExitStack

import concourse.bass as bass
import concourse.tile as tile
import concourse.tile_utils as tile_utils
from concourse import bacc, mybir
from concourse.bass_utils import run_bass_kernel_spmd

tile_utils.max_sbuf_usage = 208 * 1024

F32 = mybir.dt.float32
BF16 = mybir.dt.bfloat16
ALU = mybir.AluOpType
ACTF = mybir.ActivationFunctionType

NCORES = 8
B, V = 2048, 64
HID, TDIM = 256, 128
BC = B // NCORES            # 256 polygons / core
N = BC * V                  # 16384 nodes / core
NT = 512                    # matmul node tile
SCH = 8192                  # chunk = 128 polys (softmax poly-partition layout)
CCH = 2048                  # combine subchunk = 32 polys
GP = CCH // V               # polys per subchunk


def _ablk(asrc, atgt):
    NH, FO = asrc.shape
    out = np.zeros((NH * FO, 2 * NH), np.float32)
    for h in range(NH):
        out[h * FO:(h + 1) * FO, h] = asrc[h]
        out[h * FO:(h + 1) * FO, NH + h] = atgt[h]
    return out


def _bf(a):
    return np.ascontiguousarray(np.asarray(a, np.float32).astype(ml_dtypes.bfloat16))


def _f32(a):
    return np.ascontiguousarray(np.asarray(a, np.float32))


def half3(a):
    """(256, X) host -> (128, 2, X) so tile[:, j, :] == rows 128j:128j+128."""
    a = np.asarray(a)
    return np.ascontiguousarray(a.reshape(2, 128, a.shape[1]).transpose(1, 0, 2))


def _scb(cs, R):
    """Score-copy bias [128,1]: row 32s+r gets -colsum[r] (stacked PSUM)."""
    z = np.zeros((128, 1), np.float32)
    for s in range(4):
        z[32 * s:32 * s + 2 * R, 0] = -cs
    return z


def build(weights):
    nc = bacc.Bacc("TRN2", target_bir_lowering=False, debug=False,
                   enable_asserts=False, num_devices=NCORES)
    w = weights

    def inl(name, arr):
        return nc.inline_tensor(np.ascontiguousarray(arr), name=name).ap()

    half = TDIM // 2
    freqs = np.exp(-np.log(10000.0) * np.arange(half, dtype=np.float32) / (half - 1))
    fr2 = np.stack([np.concatenate([freqs, freqs]),
                    np.concatenate([np.zeros(half, np.float32),
                                    np.full(half, np.pi / 2, np.float32)])])
    ph = np.arange(V, dtype=np.float32) * (2 * np.pi / V)
    posT = np.tile(np.stack([np.sin(ph), np.cos(ph), np.sin(2 * ph), np.cos(2 * ph)]), (1, BC))

    # ---- host-side weight prep ----
    W0 = _f32(w["W0"]); sk0 = _f32(w["skip0"]); ab0 = _ablk(_f32(w["asrc0"]), _f32(w["atgt0"]))
    c_fr2 = inl("fr2", fr2.astype(np.float32))
    c_tW = inl("tW", _f32(w["tW"]))
    c_tb = inl("tb", _f32(w["tb"]).reshape(-1, 1))
    c_posT = inl("posT", _bf(posT))
    c_Wsum0t = inl("Wsum0t", W0[6:] + sk0[6:])          # (128, 256) f32, G0tT rhs
    c_W0ab = inl("W0ab", W0[6:] @ ab0)                  # (128, 8) f32, s0g
    c_W0p = inl("W0p", _bf(W0[:6]))                     # (6, 256)
    c_S0p = inl("S0p", _bf(sk0[:6]))                    # (6, 256)
    c_Wab0 = inl("Wab0", _bf(W0[:6] @ ab0))             # (6, 8)
    c_eye8 = inl("eye8", np.eye(8, dtype=np.float32))
    ind64 = np.kron(np.eye(64, dtype=np.float32), np.ones((1, V), np.float32))
    c_ind64 = inl("ind64", _bf(ind64))                  # (64, 4096)

    # per-layer device weights/biases
    LW, LS, LAB, LBP, LBC, LSB = {}, {}, {}, {}, {}, {}
    # L0: input is raw x0 (no ELU+1 correction)
    LW[0] = c_W0p; LS[0] = c_S0p; LAB[0] = c_Wab0
    LBP[0] = inl("bp0", half3(np.zeros((256, 1), np.float32)))
    LBC[0] = inl("bc0", half3(_f32(w["b0"]).reshape(-1, 1)))
    LSB[0] = inl("sb0", _scb(np.zeros(8, np.float32), 4))
    for i in (1, 2):
        Wi = _f32(w[f"W{i}"]); Si = _f32(w[f"skip{i}"])
        abi = _ablk(_f32(w[f"asrc{i}"]), _f32(w[f"atgt{i}"]))
        LW[i] = inl(f"W{i}h", half3(_bf(Wi)))
        LS[i] = inl(f"S{i}h", half3(_bf(Si)))
        LAB[i] = inl(f"Wab{i}", half3(_bf(Wi @ abi)))   # (128, 2, 8)
        LBP[i] = inl(f"bp{i}", half3(np.zeros((256, 1), np.float32)))
        LBC[i] = inl(f"bc{i}", half3(_f32(w[f"b{i}"]).reshape(-1, 1)))
        LSB[i] = inl(f"sb{i}", _scb(np.zeros(8, np.float32), 4))
    W3 = _f32(w["W3"]); ab3 = _ablk(_f32(w["asrc3"]), _f32(w["atgt3"]))
    LW[3] = inl("W3h", half3(_bf(W3)))
    LAB[3] = inl("Wab3", half3(_bf(W3 @ ab3)))          # (128, 2, 2)
    LBP[3] = inl("bp3", half3(_f32(w["b3"]).reshape(-1, 1)))
    LSB[3] = inl("sb3", _scb(np.zeros(2, np.float32), 1))
    c_h1W = inl("h1Wf", half3(_bf(_f32(w["h1W"]))))
    c_h1b = inl("h1bc", half3(_f32(w["h1b"]).reshape(-1, 1)))
    c_h2W = inl("h2Wf", half3(_bf(_f32(w["h2W"]))))
    c_h2b = inl("h2bc", _f32(w["h2b"]).reshape(-1, 1))

    xT = nc.dram_tensor("xT", [2, N], BF16, kind="ExternalInput").ap()
    tp = nc.dram_tensor("tp", [2, BC], F32, kind="ExternalInput").ap()
    yT = nc.dram_tensor("yT", [2, N], F32, kind="ExternalOutput").ap()

    with tile.TileContext(nc) as tc, ExitStack() as ctx:
        P = ctx.enter_context(tc.tile_pool(name="pers", bufs=1))
        WP = ctx.enter_context(tc.tile_pool(name="wts", bufs=1))
        DR = ctx.enter_context(tc.tile_pool(name="dram", bufs=2, space="DRAM"))
        PS = ctx.enter_context(tc.tile_pool(name="ps", bufs=5, space="PSUM"))
        PSC = ctx.enter_context(tc.tile_pool(name="pssc", bufs=3, space="PSUM"))
        SM = ctx.enter_context(tc.tile_pool(name="sm", bufs=1))
        CB = ctx.enter_context(tc.tile_pool(name="cb", bufs=1))
        CB2 = ctx.enter_context(tc.tile_pool(name="cb2", bufs=2))
        PJ = ctx.enter_context(tc.tile_pool(name="pj", bufs=1))

        h = P.tile([128, 2, N], BF16, tag="h")
        tembT = P.tile([TDIM, BC], F32, tag="tembT")
        G0tT = P.tile([64, 2, 2, 256], BF16, tag="G0tT")  # [g%64, ch, g//64, p]
        s0gT2 = P.tile([128, 2, 8], F32, tag="s0gT2")

        def load(c_ap, tag):
            t = WP.tile(list(c_ap.shape), c_ap.dtype, tag=tag)
            nc.sync.dma_start(t[:], c_ap)
            return t

        TT = nc.vector.tensor_tensor
        GT = nc.gpsimd.tensor_tensor
        GSTT = nc.gpsimd.scalar_tensor_tensor
        STT = nc.vector.scalar_tensor_tensor
        TS = nc.vector.tensor_scalar

        # ---- preamble: time embedding ----
        t_eye8 = load(c_eye8, "eye8")
        t_fr2 = load(c_fr2, "fr2")
        t_tp = load(tp, "tp")
        ps_te = PSC.tile([TDIM, BC], F32, tag="psA")
        nc.tensor.matmul(ps_te[:], t_fr2[:], t_tp[:], start=True, stop=True)
        te_m = SM.tile([TDIM, BC], F32, tag="Sblk")
        te_q = SM.tile([TDIM, BC], mybir.dt.int32, tag="den")
        nc.vector.tensor_scalar(te_q[:], ps_te[:], float(1.0 / (2 * np.pi)), None, op0=ALU.mult)
        te_qf = SM.tile([TDIM, BC], F32, tag="rd")
        nc.vector.tensor_copy(te_qf[:], te_q[:])
        nc.vector.scalar_tensor_tensor(te_m[:], te_qf[:], float(-2 * np.pi), ps_te[:],
                                       op0=ALU.mult, op1=ALU.add)
        te_s = SM.tile([TDIM, BC], F32, tag="E")
        nc.scalar.activation(te_s[:], te_m[:], ACTF.Sin)
        t_tW = load(c_tW, "tW")
        t_tb = load(c_tb, "tb")
        ps_tm = PSC.tile([TDIM, BC], F32, tag="psA")
        nc.tensor.matmul(ps_tm[:], t_tW[:], te_s[:], start=True, stop=True)
        nc.scalar.activation(tembT[:], ps_tm[:], ACTF.Silu, bias=t_tb[:])

        # G0tT[g, p] = sum_k tembT[k, g] * Wsum0t[k, p]   (per-graph L0 const)
        t_Ws0 = load(c_Wsum0t, "Ws0")
        for ch in range(2):
            ps_g = PSC.tile([128, 256], F32, tag="psA")
            nc.tensor.matmul(ps_g[:], tembT[:, ch * 128:(ch + 1) * 128], t_Ws0[:],
                             start=True, stop=True)
            for bd in range(2):
                nc.scalar.activation(G0tT[:, ch, bd, :], ps_g[bd * 64:(bd + 1) * 64, :],
                                     ACTF.Copy)
        # s0g: per-graph score offsets for L0, transposed to [128, 2, 8]
        t_W0ab = load(c_W0ab, "W0ab")
        ps_sg = PSC.tile([8, BC], F32, tag="psA")
        nc.tensor.matmul(ps_sg[:], t_W0ab[:], tembT[:], start=True, stop=True)
        s0g = SM.tile([8, BC], F32, tag="EX")
        nc.vector.tensor_copy(s0g[:], ps_sg[:])
        for m in range(2):
            ps_t = PSC.tile([128, 8], F32, tag="psA")
            nc.tensor.matmul(ps_t[:], s0g[:, m * 128:(m + 1) * 128], t_eye8[:],
                             is_transpose=True, start=True, stop=True)
            nc.vector.tensor_copy(s0gT2[:, m, :], ps_t[:])

        t_ind64 = load(c_ind64, "ind64")
        t_h1W = load(c_h1W, "h1W")
        t_h1b = load(c_h1b, "h1b")
        t_h2W = load(c_h2W, "h2W")
        t_h2b = load(c_h2b, "h2b")
        h0loc = PJ.tile([6, SCH], BF16, tag="h0loc")

        def layer(li):
            first = (li == 0)
            last = (li == 3)
            R = 1 if last else 4
            kt = 1 if first else 2
            t_w = load(LW[li], f"w{li}")
            t_s = None if last else load(LS[li], f"s{li}")
            t_wab = load(LAB[li], f"ab{li}")
            t_bp = load(LBP[li], f"bp{li}")
            t_bc = None if last else load(LBC[li], f"bc{li}")
            t_sb = load(LSB[li], f"sb{li}")
            a_dram = DR.tile([3 * R, N], BF16, tag="a_dram")

            def lhs(t, k, m):
                if first:
                    return t[:, m * 128:(m + 1) * 128]
                return t[:, k, m * 128:(m + 1) * 128]

            def rhs(k, u, nn):
                if first:
                    return h0loc[:, u:u + nn]
                return h[:, k, u:u + nn]

            for ch in range(N // SCH):
                u0 = ch * SCH
                if first:
                    nc.sync.dma_start(h0loc[0:2, :], xT[:, u0:u0 + SCH])
                    nc.sync.dma_start(h0loc[2:6, :], c_posT[:, u0:u0 + SCH])

                # ---- pass A: attention scores ----
                scT = SM.tile([128, SCH // 4], BF16, tag="scT")
                for it in range(SCH // NT):
                    u = it * NT
                    s = it % 4
                    if s == 0:
                        scp = PSC.tile([128, NT], F32, tag="psA")
                    for k in range(kt):
                        nc.tensor.matmul(scp[32 * s:32 * s + 2 * R, :],
                                         (t_wab[:] if first else t_wab[:, k, :]),
                                         rhs(k, u0 + u if not first else u, NT),
                                         start=(k == 0), stop=(k == kt - 1),
                                         tile_position=(0, 32 * s))
                    if s == 3:
                        g = it // 4
                        nc.scalar.activation(scT[:, g * NT:(g + 1) * NT], scp[:],
                                             ACTF.Identity, bias=t_sb[:])

                sc_dram = DR.tile([2 * R, SCH], BF16, tag="sc_dram")
                scd = sc_dram[:].rearrange("r (cb s w) -> r cb s w", s=4, w=NT)
                for s in range(4):
                    src = scT[32 * s:32 * s + 2 * R, :].rearrange("p (cb w) -> p cb w", w=NT)
                    nc.sync.dma_start(scd[:, :, s, :], src)
                S = SM.tile([128, 2 * R * V], BF16, tag="Sblk")
                nc.sync.dma_start(S[:].rearrange("p (r v) -> p r v", v=V),
                                  sc_dram[:].rearrange("r (p v) -> p r v", v=V))

                if first:
                    gb = s0gT2[:, ch, :].unsqueeze(2).to_broadcast((128, 2 * R, V))
                    Sv = S[:].rearrange("p (r v) -> p r v", v=V)
                    TT(Sv, Sv, gb, op=ALU.add)

                # E rows (by target v): [0]=from v-1, [1]=self, [2]=from v+1
                E = SM.tile([128, 3 * R * V], BF16, tag="E")
                Sv = S[:].rearrange("p (r v) -> p r v", v=V)
                Ssrc, Stgt = Sv[:, 0:R, :], Sv[:, R:2 * R, :]
                Ev = E[:].rearrange("p (k r v) -> p k r v", k=3, v=V)
                TT(Ev[:, 0, :, 1:], Ssrc[:, :, :V - 1], Stgt[:, :, 1:], op=ALU.add)
                TT(Ev[:, 0, :, 0:1], Ssrc[:, :, V - 1:], Stgt[:, :, 0:1], op=ALU.add)
                TT(Ev[:, 1, :, :], Ssrc, Stgt, op=ALU.add)
                TT(Ev[:, 2, :, :V - 1], Ssrc[:, :, 1:], Stgt[:, :, :V - 1], op=ALU.add)
                TT(Ev[:, 2, :, V - 1:], Ssrc[:, :, 0:1], Stgt[:, :, V - 1:], op=ALU.add)
                STT(E[:], E[:], 0.2, E[:], op0=ALU.mult, op1=ALU.max)
                EX = SM.tile([128, 3 * R * V], BF16, tag="EX")
                nc.scalar.activation(EX[:], E[:], ACTF.Exp)
                den = SM.tile([128, R * V], F32, tag="den")
                TT(den[:], EX[:, 0:R * V], EX[:, R * V:2 * R * V], op=ALU.add)
                TT(den[:], den[:], EX[:, 2 * R * V:], op=ALU.add)
                rd = SM.tile([128, R * V], F32, tag="rd")
                nc.vector.reciprocal(rd[:], den[:])
                ab_blk = SM.tile([128, 3 * R * V], BF16, tag="ab_blk")
                for k in range(3):
                    TT(ab_blk[:, k * R * V:(k + 1) * R * V],
                       EX[:, k * R * V:(k + 1) * R * V], rd[:], op=ALU.mult)
                nc.sync.dma_start(
                    a_dram[:, u0:u0 + SCH].rearrange("j (p v) -> p j v", v=V),
                    ab_blk[:].rearrange("p (j v) -> p j v", v=V))

                # ---- pass B: proj matmuls + combine per subchunk ----
                for sc in range(SCH // CCH):
                    v0 = sc * CCH
                    span = slice(u0 + v0, u0 + v0 + CCH)
                    # alpha broadcast: af[k][p, m, c] = alpha_k[head(m,p), c]
                    af = [CB.tile([128, 2, CCH], BF16, tag=f"af{k}", name=f"af{k}")
                          for k in range(3)]
                    for k in range(3):
                        if last:
                            for m in range(2):
                                src = a_dram[k:k + 1, span].to_broadcast((128, CCH))
                                nc.sync.dma_start(af[k][:, m, :], src)
                        else:
                            for m in range(2):
                                for hh in range(2):
                                    row = 4 * k + 2 * m + hh
                                    src = (a_dram[row:row + 1, span]
                                           .to_broadcast((64, CCH)))
                                    nc.sync.dma_start(
                                        af[k][hh * 64:(hh + 1) * 64, m, :], src)

                    Ppad = CB2.tile([128, 2, GP, V + 2], BF16, tag="Ppad")
                    Pc = None if last else CB2.tile([128, 2, CCH], BF16, tag="Pc")
                    for t in range(CCH // NT):
                        u = v0 + t * NT
                        psP = [PS.tile([128, NT], F32, tag="mm", name=f"mmp{m}")
                               for m in range(2)]
                        for m in range(2):
                            for k in range(kt):
                                nc.tensor.matmul(psP[m][:], lhs(t_w, k, m),
                                                 rhs(k, u0 + u if not first else u, NT),
                                                 start=(k == 0), stop=(k == kt - 1))
                        if not last:
                            psC = [PS.tile([128, NT], F32, tag="mm", name=f"mmc{m}")
                                   for m in range(2)]
                            for m in range(2):
                                if first:
                                    nc.tensor.matmul(psC[m][:], lhs(t_s, 0, m),
                                                     rhs(0, u, NT),
                                                     start=True, stop=False)
                                    g0 = ((u0 + u) // V) % 128
                                    band, off = g0 // 64, (g0 % 64) * V
                                    nc.tensor.matmul(
                                        psC[m][:],
                                        G0tT[:, ch, band, m * 128:(m + 1) * 128],
                                        t_ind64[:, off:off + NT],
                                        start=False, stop=True)
                                else:
                                    for k in range(kt):
                                        nc.tensor.matmul(psC[m][:], lhs(t_s, k, m),
                                                         rhs(k, u0 + u, NT),
                                                         start=(k == 0), stop=(k == kt - 1))
                        g8 = t * (NT // V)
                        for m in range(2):
                            nc.scalar.activation(
                                Ppad[:, m, g8:g8 + NT // V, 1:V + 1],
                                psP[m][:].rearrange("p (g v) -> p g v", v=V),
                                ACTF.Identity, bias=t_bp[:, m, :])
                            if not last:
                                nc.scalar.activation(Pc[:, m, t * NT:(t + 1) * NT],
                                                     psC[m][:], ACTF.Identity,
                                                     bias=t_bc[:, m, :])
                    # cyclic pads
                    nc.vector.tensor_copy(Ppad[:, :, :, 0:1], Ppad[:, :, :, V:V + 1])
                    nc.vector.tensor_copy(Ppad[:, :, :, V + 1:V + 2], Ppad[:, :, :, 1:2])

                    afv = [af[k][:].rearrange("p m (g v) -> p m g v", v=V)
                           for k in range(3)]
                    P0v = Ppad[:, :, :, 1:V + 1]
                    Pmv = Ppad[:, :, :, 0:V]
                    Ppv = Ppad[:, :, :, 2:V + 2]
                    t1 = CB.tile([128, 2, GP, V], BF16, tag="t1")
                    t2 = CB2.tile([128, 2, GP, V], BF16, tag="t2")
                    t3 = CB.tile([128, 2, GP, V], BF16, tag="t3")
                    TT(t1[:], afv[1], P0v, op=ALU.mult)             # self
                    GT(t2[:], afv[0], Pmv, op=ALU.mult)             # from v-1 (Pool)
                    TT(t3[:], afv[2], Ppv, op=ALU.mult)             # from v+1
                    TT(t1[:], t1[:], t3[:], op=ALU.add)
                    TT(t1[:], t1[:], t2[:], op=ALU.add)
                    t1f = t1[:].rearrange("p m g v -> p m (g v)")
                    if last:
                        TT(t1f, t1f, h[:, :, span], op=ALU.add)   # o3 in t1
                        # fused head: h5 = silu(h1W^T o3 + h1b); y = h2W^T h5 + h2b
                        o3 = t1f
                        h5 = t3[:].rearrange("p m g v -> p m (g v)")
                        for t in range(CCH // NT):
                            tsl = slice(t * NT, (t + 1) * NT)
                            psH = [PS.tile([128, NT], F32, tag="mm", name=f"mmh{m}")
                                   for m in range(2)]
                            for m in range(2):
                                for k in range(2):
                                    nc.tensor.matmul(psH[m][:], t_h1W[:, k, m * 128:(m + 1) * 128],
                                                     o3[:, k, tsl], start=(k == 0), stop=(k == 1))
                            for m in range(2):
                                nc.scalar.activation(h5[:, m, tsl], psH[m][:],
                                                     ACTF.Silu, bias=t_h1b[:, m, :])
                            ps2 = PSC.tile([2, NT], F32, tag="psA")
                            for k in range(2):
                                nc.tensor.matmul(ps2[:], t_h2W[:, k, :], h5[:, k, tsl],
                                                 start=(k == 0), stop=(k == 1))
                            yst = SM.tile([2, NT], F32, tag="yst")
                            TS(yst[:], ps2[:], t_h2b[:], None, op0=ALU.add)
                            nc.sync.dma_start(yT[:, u0 + v0 + t * NT:u0 + v0 + (t + 1) * NT],
                                              yst[:])
                    else:
                        TT(t1f, t1f, Pc[:], op=ALU.add)
                        # ELU: h = exp(min(x,0)) + (max(x,0) - 1)
                        TS(t3[:], t1[:], 0.0, None, op0=ALU.min)
                        nc.scalar.activation(t2[:], t3[:], ACTF.Exp)
                        TS(t3[:], t1[:], 0.0, -1.0, op0=ALU.max, op1=ALU.add)
                        eng = TT if sc % 2 == 0 else GT
                        eng(h[:, :, span], t2[:].rearrange("p m g v -> p m (g v)"),
                            t3[:].rearrange("p m g v -> p m (g v)"), op=ALU.add)

        layer(0)
        layer(1)
        layer(2)
        layer(3)

    nc.compile()
    return nc


def kernel(**inputs):
    x = np.asarray(inputs["x"], np.float32)
    t = np.asarray(inputs["t"])
    nc = build(inputs)
    in_maps = []
    for c in range(NCORES):
        xs = x[c * BC:(c + 1) * BC]
        xTs = np.ascontiguousarray(xs.reshape(N, 2).T).astype(ml_dtypes.bfloat16)
        ts = t[c * BC:(c + 1) * BC].astype(np.float32)
        tps = np.ascontiguousarray(np.stack([ts, np.ones_like(ts)]))
        in_maps.append({"xT": xTs, "tp": tps})
    res = run_bass_kernel_spmd(nc, in_maps, core_ids=list(range(NCORES)))
    outs = []
    for c in range(NCORES):
        yTs = res.results[c]["yT"]
        outs.append(yTs.T.reshape(BC, 2 * V).astype(np.float32))
    return np.concatenate(outs, 0)
